# revision 20
# baseline (speedup 1.0000x reference)
"""Trainium2 Bass kernel for CausalSelfAttention (sliding window + sink).

Sharding: 8 cores = (batch 2) x (head-group 4). Each core computes Q/K/V
for its 4 heads over the full T=2048 sequence (no halo recompute), runs
banded attention (key chunks of 128, per-512-query-block windows, sink
keys precomputed upfront), normalizes with a ones-column denominator,
projects through its 256 rows of w_proj, and returns a transposed partial
output [C, T] in bf16. The host sums the 4 per-batch partials.

All matmuls bf16 (fp32 psum). The attention loop is software-pipelined:
scores for iteration i+1 are issued before AV of iteration i, with score
psum banks double-buffered by iteration parity (tags s0..s5, yt y0/y1).
"""

import numpy as np
import ml_dtypes

B, T, C, NH, HD = 2, 2048, 1024, 16, 64
WIN, SINK = 256, 4
G, HPG, DG = 4, 4, 256   # head groups, heads/group, dims/group
NT, TCH = 4, 512         # t-chunks
NCORES = 8

BF = ml_dtypes.bfloat16

_cache = {}


def _sc_layout(qc):
    """Score/AV layout for query block qc: (kc, qoff, w, bank, boff)."""
    if qc == 0:
        sc = [
            (0, 0, 512, 0, 0),
            (1, 128, 384, 1, 0),
            (3, 384, 128, 1, 384),
            (2, 256, 256, 2, 0),
        ]
        av = [
            (0, 0, 512, 0, 0, True),
            (1, 128, 384, 1, 0, False),
            (2, 256, 256, 2, 0, False),
            (3, 384, 128, 1, 384, False),
        ]
    else:
        k0 = 4 * qc
        sc = [
            (k0, 0, 384, 0, 0),
            (k0 - 2, 0, 128, 0, 384),
            (k0 + 1, 128, 384, 1, 0),
            (k0 + 3, 384, 128, 1, 384),
            (k0 - 1, 0, 256, 2, 0),
            (k0 + 2, 256, 256, 2, 256),
        ]
        av = [
            (k0, 0, 384, 0, 0, False),
            (k0 + 3, 384, 128, 1, 384, False),
            (k0 - 2, 0, 128, 0, 384, False),
            (k0 - 1, 0, 256, 2, 0, False),
            (k0 + 1, 128, 384, 1, 0, False),
            (k0 + 2, 256, 256, 2, 256, False),
        ]
    return sc, av


EXPW = {0: [512, 512, 256], 1: [512, 512, 512]}


def _build_nc():
    import concourse.bacc as bacc
    import concourse.mybir as mybir
    import concourse.tile as tile

    f32 = mybir.dt.float32
    bf16 = mybir.dt.bfloat16
    AF = mybir.ActivationFunctionType

    nc = bacc.Bacc("TRN2", target_bir_lowering=False, debug=False,
                   num_devices=NCORES)

    xT = nc.dram_tensor("xT", [C, T], bf16, kind="ExternalInput").ap()
    wq = nc.dram_tensor("wq", [128, 2048], bf16, kind="ExternalInput").ap()
    wk = nc.dram_tensor("wk", [128, 2048], bf16, kind="ExternalInput").ap()
    wv = nc.dram_tensor("wv", [128, 2048], bf16, kind="ExternalInput").ap()
    wp = nc.dram_tensor("wp", [128, 2048], bf16, kind="ExternalInput").ap()
    cosd = nc.dram_tensor("cos", [128, T], bf16, kind="ExternalInput").ap()
    sind = nc.dram_tensor("sin", [128, T], bf16, kind="ExternalInput").ap()
    p2d = nc.dram_tensor("p2", [128, 128], bf16, kind="ExternalInput").ap()
    mask1d = nc.dram_tensor("mask1", [128, 1536], bf16,
                            kind="ExternalInput").ap()
    mask0d = nc.dram_tensor("mask0", [128, 1280], bf16,
                            kind="ExternalInput").ap()
    rseld = nc.dram_tensor("rsel", [8, 512], bf16,
                           kind="ExternalInput").ap()
    outT = nc.dram_tensor("outT", [C, T], bf16, kind="ExternalOutput").ap()

    with tile.TileContext(nc) as tc:
        with (
            tc.tile_pool(name="pers", bufs=1) as pers,
            tc.tile_pool(name="sb", bufs=1) as sb,
            tc.tile_pool(name="wk2", bufs=2) as wk2,
            tc.tile_pool(name="pr", bufs=2) as prp,
            tc.tile_pool(name="ps", bufs=1, space="PSUM") as ps,
        ):
            # ---------------- persistent loads ----------------
            wq_t = pers.tile([128, 2048], bf16, tag="wq")
            nc.sync.dma_start(wq_t[:], wq[:])
            xts = []
            for i in range(8):
                t = pers.tile([128, T], bf16, tag=f"xt{i}", name=f"xt{i}")
                nc.sync.dma_start(t[:, 0:1024], xT[i * 128:(i + 1) * 128,
                                                   0:1024])
                xts.append(t)
            wk_t = pers.tile([128, 2048], bf16, tag="wk")
            nc.sync.dma_start(wk_t[:], wk[:])
            wv_t = pers.tile([128, 2048], bf16, tag="wv")
            nc.sync.dma_start(wv_t[:], wv[:])
            tcos = pers.tile([128, T], bf16, tag="cos")
            nc.sync.dma_start(tcos[:], cosd[:])
            tsin = pers.tile([128, T], bf16, tag="sin")
            nc.sync.dma_start(tsin[:], sind[:])
            tp2 = pers.tile([128, 128], bf16, tag="p2")
            nc.sync.dma_start(tp2[:], p2d[:])
            for i in range(8):
                nc.sync.dma_start(xts[i][:, 1024:2048],
                                  xT[i * 128:(i + 1) * 128, 1024:2048])
            tmask1 = pers.tile([128, 1536], bf16, tag="mask1")
            nc.sync.dma_start(tmask1[:], mask1d[:])
            tmask0 = pers.tile([128, 1280], bf16, tag="mask0")
            nc.sync.dma_start(tmask0[:], mask0d[:])
            trsel = pers.tile([8, 512], bf16, tag="rsel")
            nc.sync.dma_start(trsel[:], rseld[:])
            wp_t = pers.tile([128, 2048], bf16, tag="wp")
            nc.sync.dma_start(wp_t[:], wp[:])

            # ---------------- QKV + RoPE ----------------
            qT = [sb.tile([128, T], bf16, tag=f"qT{d}", name=f"qT{d}")
                  for d in range(2)]
            kT = [sb.tile([128, T], bf16, tag=f"kT{d}", name=f"kT{d}")
                  for d in range(2)]
            qraw = [sb.tile([128, T], bf16, tag=f"qraw{d}", name=f"qraw{d}")
                    for d in range(2)]
            kraw = [sb.tile([128, T], bf16, tag=f"kraw{d}", name=f"kraw{d}")
                    for d in range(2)]

            def proj_stage(w_t, dt, dst_raw, nm):
                for tci in range(NT):
                    acc = ps.tile([128, 512], f32, tag=f"s{tci}",
                                  name=f"acc_{nm}{dt}_{tci}")
                    for kc in range(8):
                        nc.tensor.matmul(
                            acc[:], w_t[:, (kc * 2 + dt) * 128:
                                        (kc * 2 + dt + 1) * 128],
                            xts[kc][:, tci * 512:(tci + 1) * 512],
                            start=(kc == 0), stop=(kc == 7),
                        )
                    nc.vector.tensor_copy(
                        dst_raw[:, tci * 512:(tci + 1) * 512], acc[:])

            def rope_stage(raw, dst, nm):
                for tci in range(NT):
                    sl = slice(tci * 512, (tci + 1) * 512)
                    prot = ps.tile([128, 512], f32, tag=f"s{4 + tci % 2}",
                                   name=f"rot_{nm}_{tci}")
                    nc.tensor.matmul(prot[:], tp2[:], raw[:, sl],
                                     start=True, stop=True)
                    tmp = wk2.tile([128, 512], bf16, tag="ropetmp",
                                   name=f"rt_{nm}_{tci}")
                    nc.vector.tensor_mul(tmp[:], prot[:], tsin[:, sl])
                    nc.gpsimd.tensor_mul(dst[:, sl], raw[:, sl], tcos[:, sl])
                    nc.vector.tensor_add(dst[:, sl], dst[:, sl], tmp[:])

            for dt in range(2):
                proj_stage(wq_t, dt, qraw[dt], "q")
                rope_stage(qraw[dt], qT[dt], f"q{dt}")
            for dt in range(2):
                proj_stage(wk_t, dt, kraw[dt], "k")
                rope_stage(kraw[dt], kT[dt], f"k{dt}")

            # V in [keys, d] layout with ones column
            vsb = []
            for kt in range(16):
                pv = ps.tile([128, 512], f32, tag=f"s{kt % 4}",
                             name=f"pv{kt}")
                for kc in range(8):
                    nc.tensor.matmul(
                        pv[:, 0:256], xts[kc][:, kt * 128:(kt + 1) * 128],
                        wv_t[:, kc * 256:(kc + 1) * 256],
                        start=(kc == 0), stop=(kc == 7),
                    )
                vt = sb.tile([128, 260], bf16, tag=f"v{kt}", name=f"v{kt}")
                vr = vt.rearrange("p (h e) -> p h e", e=65)
                nc.vector.tensor_copy(
                    vr[:, :, 0:64],
                    pv[:, 0:256].rearrange("p (h e) -> p h e", e=64))
                nc.gpsimd.memset(vr[:, :, 64:65], 1.0)
                vsb.append(vt)

            v_sink = []
            for p in range(2):
                vs = sb.tile([128, 65], bf16, tag=f"vsink{p}",
                             name=f"vsink{p}")
                nc.gpsimd.memset(vs[:], 0.0)
                for e in range(2):
                    h = 2 * p + e
                    nc.scalar.copy(vs[64 * e:64 * e + 4, 0:65],
                                   vsb[0][0:4, 65 * h:65 * h + 65])
                v_sink.append(vs)

            # ---------------- sink scores upfront ----------------
            # snkpr[qc][p]: heads (2p, 2p+1) at strips 0/64, exp'd probs
            snkpr = {}
            for qc in range(1, 4):
                for p in range(2):
                    snk = ps.tile([128, 512], f32,
                                  tag=f"s{(2 * qc + p) % 6}",
                                  name=f"snk{qc}_{p}")
                    for e in range(2):
                        h = 2 * p + e
                        dtile = h // 2
                        dsl = slice((h % 2) * 64, (h % 2) * 64 + 64)
                        nc.tensor.matmul(
                            snk[64 * e:64 * e + 4, :],
                            kT[dtile][dsl, 0:4],
                            qT[dtile][dsl, qc * 512:(qc + 1) * 512],
                            start=True, stop=True,
                        )
                    sp = sb.tile([128, 512], bf16, tag=f"snkpr{qc}_{p}",
                                 name=f"snkpr{qc}_{p}")
                    nc.scalar.activation(sp[0:68, :], snk[0:68, :],
                                         AF.Exp, scale=0.125)
                    snkpr[(qc, p)] = sp

            # ---------------- pipelined attention ----------------
            stg = {}   # (h, qc) -> [65, 512] f32 sbuf (rows 0-63 y, 64 den)
            dn8 = [sb.tile([8, 512], f32, tag=f"dn8{s}", name=f"dn8{s}")
                   for s in range(2)]
            ytu = [sb.tile([128, T], bf16, tag=f"ytu{d}", name=f"ytu{d}")
                   for d in range(2)]

            def emit_scores(i, qc, h):
                dtile = h // 2
                dsl = slice((h % 2) * 64, (h % 2) * 64 + 64)
                sc_l, _ = _sc_layout(qc)
                par = 3 * (i % 2)
                scb = [ps.tile([128, 512], f32, tag=f"s{par + b}",
                               name=f"sc{qc}_{h}_{b}") for b in range(3)]
                for (kc, qoff, w, bank, boff) in sc_l:
                    nc.tensor.matmul(
                        scb[bank][:, boff:boff + w],
                        kT[dtile][dsl, kc * 128:(kc + 1) * 128],
                        qT[dtile][dsl, qc * 512 + qoff:qc * 512 + qoff + w],
                        start=True, stop=True,
                    )
                probs = prp.tile([128, 1536], bf16, tag="pr",
                                 name=f"pr{qc}_{h}")
                for b in range(3):
                    wb = EXPW[min(qc, 1)][b]
                    nc.scalar.activation(probs[:, 512 * b:512 * b + wb],
                                         scb[b][:, 0:wb],
                                         AF.Exp, scale=0.125)
                tm, tw = (tmask0, 1280) if qc == 0 else (tmask1, 1536)
                nc.vector.tensor_mul(probs[:, 0:tw], probs[:, 0:tw],
                                     tm[:, 0:tw])
                return probs

            def emit_av(i, qc, h, probs):
                _, av_l = _sc_layout(qc)
                yt = ps.tile([128, 512], f32, tag=f"y{i % 2}",
                             name=f"yt{qc}_{h}")
                if qc >= 1:
                    p, e = h // 2, h % 2
                    nc.tensor.matmul(
                        yt[0:65, :],
                        v_sink[p][64 * e:64 * e + 4, 0:65],
                        snkpr[(qc, p)][64 * e:64 * e + 4, :],
                        start=True, stop=False,
                    )
                nmm = len(av_l)
                for ii, (kc, qoff, w, bank, boff, st) in enumerate(av_l):
                    nc.tensor.matmul(
                        yt[0:65, qoff:qoff + w],
                        vsb[kc][:, 65 * h:65 * h + 65],
                        probs[:, 512 * bank + boff:512 * bank + boff + w],
                        start=st, stop=(ii == nmm - 1),
                    )
                st65 = sb.tile([65, 512], f32, tag=f"stg{qc}_{h}",
                               name=f"stg{qc}_{h}")
                nc.vector.tensor_copy(st65[:], yt[0:65, :])
                s2 = qc // 2
                nc.sync.dma_start(
                    dn8[s2][4 * (qc % 2) + h:4 * (qc % 2) + h + 1, :],
                    st65[64:65, :])
                stg[(h, qc)] = st65

            def emit_norm_proj(stage, tagbase):
                # normalize qc in {2*stage, 2*stage+1}, project those tci
                r8 = sb.tile([8, 512], bf16, tag=f"r8{stage}",
                             name=f"r8{stage}")
                with nc.allow_low_precision(reason="bf16 recip"):
                    nc.vector.reciprocal(r8[:], dn8[stage][:])
                n = 0
                for qcp in range(2):
                    qc = 2 * stage + qcp
                    for h in range(4):
                        r = 4 * qcp + h
                        dtile = h // 2
                        prb = ps.tile([128, 512], f32,
                                      tag=f"s{(tagbase + n) % 6}",
                                      name=f"prb{qc}_{h}")
                        n += 1
                        nc.tensor.matmul(prb[0:64, :],
                                         trsel[:, r * 64:(r + 1) * 64],
                                         r8[:], start=True, stop=True)
                        nc.vector.tensor_mul(
                            ytu[dtile][(h % 2) * 64:(h % 2) * 64 + 64,
                                       qc * 512:(qc + 1) * 512],
                            stg[(h, qc)][0:64, :], prb[0:64, :])
                for qcp in range(2):
                    tci = 2 * stage + qcp
                    for cc in range(8):
                        po = ps.tile([128, 512], f32,
                                     tag=f"s{(tagbase + n) % 6}",
                                     name=f"po{cc}_{tci}")
                        n += 1
                        for dt in range(2):
                            nc.tensor.matmul(
                                po[:], wp_t[:, (dt * 8 + cc) * 128:
                                            (dt * 8 + cc + 1) * 128],
                                ytu[dt][:, tci * 512:(tci + 1) * 512],
                                start=(dt == 0), stop=(dt == 1),
                            )
                        osb = prp.tile([128, 512], bf16, tag="osb",
                                       name=f"osb{cc}_{tci}")
                        if cc % 2 == 0:
                            nc.vector.tensor_copy(osb[:], po[:])
                        else:
                            nc.scalar.copy(osb[:], po[:])
                        nc.gpsimd.dma_start(
                            outT[cc * 128:(cc + 1) * 128,
                                 tci * 512:(tci + 1) * 512], osb[:])

            iters = [(qc, h) for qc in range(4) for h in range(4)]
            pend = None
            for i, (qc, h) in enumerate(iters):
                probs = emit_scores(i, qc, h)
                if pend is not None:
                    emit_av(*pend)
                if i == 9:
                    # qc 0-1 fully normalized+projected; fills PE gaps
                    # while Act/DVE churn attention iters 9-15
                    emit_norm_proj(0, 3)
                pend = (i, qc, h, probs)
            emit_av(*pend)
            emit_norm_proj(1, 0)

    nc.compile()
    return nc


def _host_inputs(x, w_attn, w_proj):
    """Build the 8 per-core input maps (core = 4*b + g)."""
    inv_freq = 1.0 / (10000.0 ** (np.arange(0, HD, 2, dtype=np.float32) / HD))
    iff = np.concatenate([inv_freq, inv_freq])  # [64]
    tpos = np.arange(T, dtype=np.float32)
    ang = tpos[None, :] * iff[:, None]            # [64, T]
    cos1 = np.cos(ang).astype(np.float32)
    sin1 = np.sin(ang).astype(np.float32)
    cos_t = np.concatenate([cos1, cos1], 0)       # [128, T]
    sin_t = np.concatenate([sin1, sin1], 0)

    P2 = np.zeros((128, 128), np.float32)
    for blk in range(2):
        o = blk * 64
        for d in range(32):
            P2[o + d + 32, o + d] = -1.0
            P2[o + d, o + d + 32] = 1.0

    k = np.arange(128)[:, None]
    j = np.arange(128)[None, :]
    diag = (j >= k).astype(np.float32)
    tail = (j < k).astype(np.float32)
    ones = np.ones((128, 128), np.float32)

    jj = np.arange(512)[None, :]
    kc0sp = ((jj >= k) & ((jj - k < WIN) | (k < SINK))).astype(np.float32)

    gen384 = np.concatenate([diag, ones, tail], 1)
    mask1 = np.concatenate(
        [gen384, tail, gen384, diag, ones, tail, diag, ones], 1)
    mask0 = np.concatenate([kc0sp, gen384, diag, diag, ones], 1)

    # rsel8: selector for (qc-parity, h) block r: one-hot row r, 64 wide
    rsel8 = np.zeros((8, 512), np.float32)
    for r in range(8):
        rsel8[r, r * 64:(r + 1) * 64] = 1.0

    def pack_lhsT(w):
        return np.ascontiguousarray(
            w.reshape(8, 128, 2, 128).transpose(1, 0, 2, 3).reshape(128, 2048))

    def pack_rhs(w):
        return np.ascontiguousarray(
            w.reshape(8, 128, 256).transpose(1, 0, 2).reshape(128, 2048))

    def pack_wp(w):
        return np.ascontiguousarray(
            w.reshape(2, 128, 8, 128).transpose(1, 0, 2, 3).reshape(128, 2048))

    bfc = lambda a: np.ascontiguousarray(a.astype(BF))

    xTb = [bfc(x[b].T) for b in range(B)]
    in_maps = []
    for core in range(NCORES):
        b, g = core // 4, core % 4
        csl = slice(g * DG, (g + 1) * DG)
        in_maps.append({
            "xT": xTb[b],
            "wq": bfc(pack_lhsT(w_attn[:, 0 * C:1 * C][:, csl])),
            "wk": bfc(pack_lhsT(w_attn[:, 1 * C:2 * C][:, csl])),
            "wv": bfc(pack_rhs(w_attn[:, 2 * C:3 * C][:, csl])),
            "wp": bfc(pack_wp(w_proj[csl, :])),
            "cos": bfc(cos_t), "sin": bfc(sin_t), "p2": bfc(P2),
            "mask1": bfc(mask1), "mask0": bfc(mask0), "rsel": bfc(rsel8),
        })
    return in_maps


def kernel(x, w_attn, w_proj):
    from concourse import bass_utils

    x = np.asarray(x, np.float32)
    w_attn = np.asarray(w_attn, np.float32)
    w_proj = np.asarray(w_proj, np.float32)

    if "nc" not in _cache:
        _cache["nc"] = _build_nc()
    nc = _cache["nc"]

    in_maps = _host_inputs(x, w_attn, w_proj)
    res = bass_utils.run_bass_kernel_spmd(nc, in_maps, list(range(NCORES)),
                                          **_cache.get("run_kwargs", {}))
    _cache["last_result"] = res

    y = np.zeros((B, T, C), np.float32)
    for core in range(NCORES):
        b = core // 4
        y[b] += res.results[core]["outT"].T.astype(np.float32)
    return y


# revision 23
# speedup vs baseline: 1.0292x; 1.0292x over previous
"""Trainium2 Bass kernel for CausalSelfAttention (sliding window + sink).

Sharding: 8 cores = (batch 2) x (head-group 4). Each core computes Q/K/V
for its 4 heads over the full T=2048 sequence (no halo recompute), runs
banded attention (key chunks of 128, per-512-query-block windows, sink
keys precomputed upfront), normalizes with a ones-column denominator,
projects through its 256 rows of w_proj, and returns a transposed partial
output [C, T] in bf16. The host sums the 4 per-batch partials.

All matmuls bf16 (fp32 psum). The attention loop is software-pipelined:
scores for iteration i+1 are issued before AV of iteration i, with score
psum banks double-buffered by iteration parity (tags s0..s5, yt y0/y1).
"""

import numpy as np
import ml_dtypes

B, T, C, NH, HD = 2, 2048, 1024, 16, 64
WIN, SINK = 256, 4
G, HPG, DG = 4, 4, 256   # head groups, heads/group, dims/group
NT, TCH = 4, 512         # t-chunks
NCORES = 8

BF = ml_dtypes.bfloat16

_cache = {}


def _sc_layout(qc):
    """Score/AV layout for query block qc: (kc, qoff, w, bank, boff)."""
    if qc == 0:
        sc = [
            (0, 0, 512, 0, 0),
            (1, 128, 384, 1, 0),
            (3, 384, 128, 1, 384),
            (2, 256, 256, 2, 0),
        ]
        av = [
            (0, 0, 512, 0, 0, True),
            (1, 128, 384, 1, 0, False),
            (2, 256, 256, 2, 0, False),
            (3, 384, 128, 1, 384, False),
        ]
    else:
        k0 = 4 * qc
        sc = [
            (k0, 0, 384, 0, 0),
            (k0 - 2, 0, 128, 0, 384),
            (k0 + 1, 128, 384, 1, 0),
            (k0 + 3, 384, 128, 1, 384),
            (k0 - 1, 0, 256, 2, 0),
            (k0 + 2, 256, 256, 2, 256),
        ]
        av = [
            (k0, 0, 384, 0, 0, False),
            (k0 + 3, 384, 128, 1, 384, False),
            (k0 - 2, 0, 128, 0, 384, False),
            (k0 - 1, 0, 256, 2, 0, False),
            (k0 + 1, 128, 384, 1, 0, False),
            (k0 + 2, 256, 256, 2, 256, False),
        ]
    return sc, av


EXPW = {0: [512, 512, 256], 1: [512, 512, 512]}


def _build_nc():
    import concourse.bacc as bacc
    import concourse.mybir as mybir
    import concourse.tile as tile

    f32 = mybir.dt.float32
    bf16 = mybir.dt.bfloat16
    AF = mybir.ActivationFunctionType

    nc = bacc.Bacc("TRN2", target_bir_lowering=False, debug=False,
                   num_devices=NCORES)

    xT = nc.dram_tensor("xT", [C, T], bf16, kind="ExternalInput").ap()
    wq = nc.dram_tensor("wq", [128, 2048], bf16, kind="ExternalInput").ap()
    wk = nc.dram_tensor("wk", [128, 2048], bf16, kind="ExternalInput").ap()
    wv = nc.dram_tensor("wv", [128, 2048], bf16, kind="ExternalInput").ap()
    wp = nc.dram_tensor("wp", [128, 2048], bf16, kind="ExternalInput").ap()
    cosd = nc.dram_tensor("cos", [128, T], bf16, kind="ExternalInput").ap()
    sind = nc.dram_tensor("sin", [128, T], bf16, kind="ExternalInput").ap()
    p2d = nc.dram_tensor("p2", [128, 128], bf16, kind="ExternalInput").ap()
    mask1d = nc.dram_tensor("mask1", [128, 1536], bf16,
                            kind="ExternalInput").ap()
    mask0d = nc.dram_tensor("mask0", [128, 1280], bf16,
                            kind="ExternalInput").ap()
    rseld = nc.dram_tensor("rsel", [8, 512], bf16,
                           kind="ExternalInput").ap()
    outT = nc.dram_tensor("outT", [C, T], bf16, kind="ExternalOutput").ap()

    with tile.TileContext(nc) as tc:
        with (
            tc.tile_pool(name="pers", bufs=1) as pers,
            tc.tile_pool(name="sb", bufs=1) as sb,
            tc.tile_pool(name="wk2", bufs=2) as wk2,
            tc.tile_pool(name="pr", bufs=2) as prp,
            tc.tile_pool(name="ps", bufs=1, space="PSUM") as ps,
        ):
            # ---------------- persistent loads ----------------
            wq_t = pers.tile([128, 2048], bf16, tag="wq")
            nc.sync.dma_start(wq_t[:], wq[:])
            xts = []
            for i in range(8):
                t = pers.tile([128, T], bf16, tag=f"xt{i}", name=f"xt{i}")
                nc.sync.dma_start(t[:, 0:1024], xT[i * 128:(i + 1) * 128,
                                                   0:1024])
                xts.append(t)
            wk_t = pers.tile([128, 2048], bf16, tag="wk")
            nc.sync.dma_start(wk_t[:], wk[:])
            wv_t = pers.tile([128, 2048], bf16, tag="wv")
            nc.sync.dma_start(wv_t[:], wv[:])
            tcos = pers.tile([128, T], bf16, tag="cos")
            nc.sync.dma_start(tcos[:], cosd[:])
            tsin = pers.tile([128, T], bf16, tag="sin")
            nc.sync.dma_start(tsin[:], sind[:])
            tp2 = pers.tile([128, 128], bf16, tag="p2")
            nc.sync.dma_start(tp2[:], p2d[:])
            for i in range(8):
                nc.sync.dma_start(xts[i][:, 1024:2048],
                                  xT[i * 128:(i + 1) * 128, 1024:2048])
            tmask1 = pers.tile([128, 1536], bf16, tag="mask1")
            nc.sync.dma_start(tmask1[:], mask1d[:])
            tmask0 = pers.tile([128, 1280], bf16, tag="mask0")
            nc.sync.dma_start(tmask0[:], mask0d[:])
            trsel = pers.tile([8, 512], bf16, tag="rsel")
            nc.sync.dma_start(trsel[:], rseld[:])
            wp_t = pers.tile([128, 2048], bf16, tag="wp")
            nc.sync.dma_start(wp_t[:], wp[:])

            # ---------------- QKV + RoPE ----------------
            qT = [sb.tile([128, T], bf16, tag=f"qT{d}", name=f"qT{d}")
                  for d in range(2)]
            kT = [sb.tile([128, T], bf16, tag=f"kT{d}", name=f"kT{d}")
                  for d in range(2)]
            qraw = [sb.tile([128, T], bf16, tag=f"qraw{d}", name=f"qraw{d}")
                    for d in range(2)]
            kraw = [sb.tile([128, T], bf16, tag=f"kraw{d}", name=f"kraw{d}")
                    for d in range(2)]

            def proj_stage(w_t, dt, dst_raw, nm):
                for tci in range(NT):
                    acc = ps.tile([128, 512], f32, tag=f"s{tci}",
                                  name=f"acc_{nm}{dt}_{tci}")
                    for kc in range(8):
                        nc.tensor.matmul(
                            acc[:], w_t[:, (kc * 2 + dt) * 128:
                                        (kc * 2 + dt + 1) * 128],
                            xts[kc][:, tci * 512:(tci + 1) * 512],
                            start=(kc == 0), stop=(kc == 7),
                        )
                    nc.vector.tensor_copy(
                        dst_raw[:, tci * 512:(tci + 1) * 512], acc[:])

            def rope_stage(raw, dst, nm):
                for tci in range(NT):
                    sl = slice(tci * 512, (tci + 1) * 512)
                    prot = ps.tile([128, 512], f32, tag=f"s{4 + tci % 2}",
                                   name=f"rot_{nm}_{tci}")
                    nc.tensor.matmul(prot[:], tp2[:], raw[:, sl],
                                     start=True, stop=True)
                    tmp = wk2.tile([128, 512], bf16, tag="ropetmp",
                                   name=f"rt_{nm}_{tci}")
                    nc.vector.tensor_mul(tmp[:], prot[:], tsin[:, sl])
                    nc.gpsimd.tensor_mul(dst[:, sl], raw[:, sl], tcos[:, sl])
                    nc.gpsimd.tensor_add(dst[:, sl], dst[:, sl], tmp[:])

            for dt in range(2):
                proj_stage(wq_t, dt, qraw[dt], "q")
                rope_stage(qraw[dt], qT[dt], f"q{dt}")
            for dt in range(2):
                proj_stage(wk_t, dt, kraw[dt], "k")
                rope_stage(kraw[dt], kT[dt], f"k{dt}")

            # V in [keys, d] layout with ones column
            vsb = []
            for kt in range(16):
                pv = ps.tile([128, 512], f32, tag=f"s{kt % 4}",
                             name=f"pv{kt}")
                for kc in range(8):
                    nc.tensor.matmul(
                        pv[:, 0:256], xts[kc][:, kt * 128:(kt + 1) * 128],
                        wv_t[:, kc * 256:(kc + 1) * 256],
                        start=(kc == 0), stop=(kc == 7),
                    )
                vt = sb.tile([128, 260], bf16, tag=f"v{kt}", name=f"v{kt}")
                vr = vt.rearrange("p (h e) -> p h e", e=65)
                nc.scalar.copy(
                    vr[:, :, 0:64],
                    pv[:, 0:256].rearrange("p (h e) -> p h e", e=64))
                nc.gpsimd.memset(vr[:, :, 64:65], 1.0)
                vsb.append(vt)

            v_sink = []
            for p in range(2):
                vs = sb.tile([128, 65], bf16, tag=f"vsink{p}",
                             name=f"vsink{p}")
                nc.gpsimd.memset(vs[:], 0.0)
                for e in range(2):
                    h = 2 * p + e
                    nc.scalar.copy(vs[64 * e:64 * e + 4, 0:65],
                                   vsb[0][0:4, 65 * h:65 * h + 65])
                v_sink.append(vs)

            # ---------------- sink scores upfront ----------------
            # snkpr[qc][p]: heads (2p, 2p+1) at strips 0/64, exp'd probs
            snkpr = {}
            for qc in range(1, 4):
                for p in range(2):
                    snk = ps.tile([128, 512], f32,
                                  tag=f"s{(2 * qc + p) % 6}",
                                  name=f"snk{qc}_{p}")
                    for e in range(2):
                        h = 2 * p + e
                        dtile = h // 2
                        dsl = slice((h % 2) * 64, (h % 2) * 64 + 64)
                        nc.tensor.matmul(
                            snk[64 * e:64 * e + 4, :],
                            kT[dtile][dsl, 0:4],
                            qT[dtile][dsl, qc * 512:(qc + 1) * 512],
                            start=True, stop=True,
                        )
                    sp = sb.tile([128, 512], bf16, tag=f"snkpr{qc}_{p}",
                                 name=f"snkpr{qc}_{p}")
                    nc.scalar.activation(sp[0:68, :], snk[0:68, :],
                                         AF.Exp, scale=0.125)
                    snkpr[(qc, p)] = sp

            # ---------------- pipelined attention ----------------
            stg = {}   # (h, qc) -> [65, 512] f32 sbuf (rows 0-63 y, 64 den)
            dn8 = [sb.tile([8, 512], f32, tag=f"dn8{s}", name=f"dn8{s}")
                   for s in range(2)]
            ytu = [sb.tile([128, T], bf16, tag=f"ytu{d}", name=f"ytu{d}")
                   for d in range(2)]

            def emit_scores(i, qc, h):
                dtile = h // 2
                dsl = slice((h % 2) * 64, (h % 2) * 64 + 64)
                sc_l, _ = _sc_layout(qc)
                par = 3 * (i % 2)
                scb = [ps.tile([128, 512], f32, tag=f"s{par + b}",
                               name=f"sc{qc}_{h}_{b}") for b in range(3)]
                for (kc, qoff, w, bank, boff) in sc_l:
                    nc.tensor.matmul(
                        scb[bank][:, boff:boff + w],
                        kT[dtile][dsl, kc * 128:(kc + 1) * 128],
                        qT[dtile][dsl, qc * 512 + qoff:qc * 512 + qoff + w],
                        start=True, stop=True,
                    )
                probs = prp.tile([128, 1536], bf16, tag="pr",
                                 name=f"pr{qc}_{h}")
                for b in range(3):
                    wb = EXPW[min(qc, 1)][b]
                    nc.scalar.activation(probs[:, 512 * b:512 * b + wb],
                                         scb[b][:, 0:wb],
                                         AF.Exp, scale=0.125)
                tm, tw = (tmask0, 1280) if qc == 0 else (tmask1, 1536)
                nc.vector.tensor_mul(probs[:, 0:tw], probs[:, 0:tw],
                                     tm[:, 0:tw])
                return probs

            def emit_av(i, qc, h, probs):
                _, av_l = _sc_layout(qc)
                yt = ps.tile([128, 512], f32, tag=f"y{i % 2}",
                             name=f"yt{qc}_{h}")
                if qc >= 1:
                    p, e = h // 2, h % 2
                    nc.tensor.matmul(
                        yt[0:65, :],
                        v_sink[p][64 * e:64 * e + 4, 0:65],
                        snkpr[(qc, p)][64 * e:64 * e + 4, :],
                        start=True, stop=False,
                    )
                nmm = len(av_l)
                for ii, (kc, qoff, w, bank, boff, st) in enumerate(av_l):
                    nc.tensor.matmul(
                        yt[0:65, qoff:qoff + w],
                        vsb[kc][:, 65 * h:65 * h + 65],
                        probs[:, 512 * bank + boff:512 * bank + boff + w],
                        start=st, stop=(ii == nmm - 1),
                    )
                st65 = sb.tile([65, 512], f32, tag=f"stg{qc}_{h}",
                               name=f"stg{qc}_{h}")
                if h % 2 == 0:
                    nc.scalar.copy(st65[:], yt[0:65, :])
                else:
                    nc.vector.tensor_copy(st65[:], yt[0:65, :])
                s2 = qc // 2
                nc.sync.dma_start(
                    dn8[s2][4 * (qc % 2) + h:4 * (qc % 2) + h + 1, :],
                    st65[64:65, :])
                stg[(h, qc)] = st65

            def emit_norm_proj(stage, tagbase):
                # normalize qc in {2*stage, 2*stage+1}, project those tci
                r8 = sb.tile([8, 512], bf16, tag=f"r8{stage}",
                             name=f"r8{stage}")
                with nc.allow_low_precision(reason="bf16 recip"):
                    nc.vector.reciprocal(r8[:], dn8[stage][:])
                n = 0
                for qcp in range(2):
                    qc = 2 * stage + qcp
                    for h in range(4):
                        r = 4 * qcp + h
                        dtile = h // 2
                        prb = ps.tile([128, 512], f32,
                                      tag=f"s{(tagbase + n) % 6}",
                                      name=f"prb{qc}_{h}")
                        n += 1
                        nc.tensor.matmul(prb[0:64, :],
                                         trsel[:, r * 64:(r + 1) * 64],
                                         r8[:], start=True, stop=True)
                        nc.vector.tensor_mul(
                            ytu[dtile][(h % 2) * 64:(h % 2) * 64 + 64,
                                       qc * 512:(qc + 1) * 512],
                            stg[(h, qc)][0:64, :], prb[0:64, :])
                for qcp in range(2):
                    tci = 2 * stage + qcp
                    for cc in range(8):
                        po = ps.tile([128, 512], f32,
                                     tag=f"s{(tagbase + n) % 6}",
                                     name=f"po{cc}_{tci}")
                        n += 1
                        for dt in range(2):
                            nc.tensor.matmul(
                                po[:], wp_t[:, (dt * 8 + cc) * 128:
                                            (dt * 8 + cc + 1) * 128],
                                ytu[dt][:, tci * 512:(tci + 1) * 512],
                                start=(dt == 0), stop=(dt == 1),
                            )
                        osb = prp.tile([128, 512], bf16, tag="osb",
                                       name=f"osb{cc}_{tci}")
                        if cc % 2 == 0:
                            nc.vector.tensor_copy(osb[:], po[:])
                        else:
                            nc.scalar.copy(osb[:], po[:])
                        nc.gpsimd.dma_start(
                            outT[cc * 128:(cc + 1) * 128,
                                 tci * 512:(tci + 1) * 512], osb[:])

            iters = [(qc, h) for qc in range(4) for h in range(4)]
            pend = None
            for i, (qc, h) in enumerate(iters):
                probs = emit_scores(i, qc, h)
                if pend is not None:
                    emit_av(*pend)
                if i == 9:
                    # qc 0-1 fully normalized+projected; fills PE gaps
                    # while Act/DVE churn attention iters 9-15
                    emit_norm_proj(0, 3)
                pend = (i, qc, h, probs)
            emit_av(*pend)
            emit_norm_proj(1, 0)

    nc.compile()
    return nc


def _host_inputs(x, w_attn, w_proj):
    """Build the 8 per-core input maps (core = 4*b + g)."""
    inv_freq = 1.0 / (10000.0 ** (np.arange(0, HD, 2, dtype=np.float32) / HD))
    iff = np.concatenate([inv_freq, inv_freq])  # [64]
    tpos = np.arange(T, dtype=np.float32)
    ang = tpos[None, :] * iff[:, None]            # [64, T]
    cos1 = np.cos(ang).astype(np.float32)
    sin1 = np.sin(ang).astype(np.float32)
    cos_t = np.concatenate([cos1, cos1], 0)       # [128, T]
    sin_t = np.concatenate([sin1, sin1], 0)

    P2 = np.zeros((128, 128), np.float32)
    for blk in range(2):
        o = blk * 64
        for d in range(32):
            P2[o + d + 32, o + d] = -1.0
            P2[o + d, o + d + 32] = 1.0

    k = np.arange(128)[:, None]
    j = np.arange(128)[None, :]
    diag = (j >= k).astype(np.float32)
    tail = (j < k).astype(np.float32)
    ones = np.ones((128, 128), np.float32)

    jj = np.arange(512)[None, :]
    kc0sp = ((jj >= k) & ((jj - k < WIN) | (k < SINK))).astype(np.float32)

    gen384 = np.concatenate([diag, ones, tail], 1)
    mask1 = np.concatenate(
        [gen384, tail, gen384, diag, ones, tail, diag, ones], 1)
    mask0 = np.concatenate([kc0sp, gen384, diag, diag, ones], 1)

    # rsel8: selector for (qc-parity, h) block r: one-hot row r, 64 wide
    rsel8 = np.zeros((8, 512), np.float32)
    for r in range(8):
        rsel8[r, r * 64:(r + 1) * 64] = 1.0

    def pack_lhsT(w):
        return np.ascontiguousarray(
            w.reshape(8, 128, 2, 128).transpose(1, 0, 2, 3).reshape(128, 2048))

    def pack_rhs(w):
        return np.ascontiguousarray(
            w.reshape(8, 128, 256).transpose(1, 0, 2).reshape(128, 2048))

    def pack_wp(w):
        return np.ascontiguousarray(
            w.reshape(2, 128, 8, 128).transpose(1, 0, 2, 3).reshape(128, 2048))

    bfc = lambda a: np.ascontiguousarray(a.astype(BF))

    xTb = [bfc(x[b].T) for b in range(B)]
    in_maps = []
    for core in range(NCORES):
        b, g = core // 4, core % 4
        csl = slice(g * DG, (g + 1) * DG)
        in_maps.append({
            "xT": xTb[b],
            "wq": bfc(pack_lhsT(w_attn[:, 0 * C:1 * C][:, csl])),
            "wk": bfc(pack_lhsT(w_attn[:, 1 * C:2 * C][:, csl])),
            "wv": bfc(pack_rhs(w_attn[:, 2 * C:3 * C][:, csl])),
            "wp": bfc(pack_wp(w_proj[csl, :])),
            "cos": bfc(cos_t), "sin": bfc(sin_t), "p2": bfc(P2),
            "mask1": bfc(mask1), "mask0": bfc(mask0), "rsel": bfc(rsel8),
        })
    return in_maps


def kernel(x, w_attn, w_proj):
    from concourse import bass_utils

    x = np.asarray(x, np.float32)
    w_attn = np.asarray(w_attn, np.float32)
    w_proj = np.asarray(w_proj, np.float32)

    if "nc" not in _cache:
        _cache["nc"] = _build_nc()
    nc = _cache["nc"]

    in_maps = _host_inputs(x, w_attn, w_proj)
    res = bass_utils.run_bass_kernel_spmd(nc, in_maps, list(range(NCORES)),
                                          **_cache.get("run_kwargs", {}))
    _cache["last_result"] = res

    y = np.zeros((B, T, C), np.float32)
    for core in range(NCORES):
        b = core // 4
        y[b] += res.results[core]["outT"].T.astype(np.float32)
    return y
